# revision 1
# baseline (speedup 1.0000x reference)
"""Trainium2 Bass kernel for nn_MinimalGazeEncoder.

Data-parallel over batch: 8 cores x 8 batch elements each.

Per-core layout: partition p = b*16 + c over 128 chunks of 512 timesteps
(b in [0,8), c in [0,16)).  A single SBUF tensor P[128, 32*512] holds one
[128, 512] "plane" per feature channel (slots 0..19 feature channels in
reference order, slot 20 = ones for the b1 bias row, 21.. scratch).

Features are computed with full-width DVE/ACT plane ops.  Time-shift chunk
boundaries (causal diff) and the EMA chunk carries are handled with a small
constant shift matrix on the PE (shift by one chunk within each batch
element); the EMA itself is a hardware prefix scan (tensor_tensor_scan)
plus a rank-1 alpha-powers carry fixup (alpha^512 underflows so carries
never chain).

Matmuls: G-tiles [128, 512] are built from P with one cast+reshape SWDGE
DMA per tile (4 chunks x 32-slot groups on the partition axis); layer 1
runs as 4 concurrent row-tiled K=21 matmuls (W1|b1 replicated at
partitions 0/32/64/96); gelu reads 4 PSUM banks as one FD=2048 ACT op;
layer 2 is flipped (lhsT = h1 block, rhs = W2) so the output lands as
[t, d] and can be DMA'd out contiguously; b2 is added with a K=1 fill
matmul into PSUM before the accumulating block matmuls.
"""

import math

import numpy as np
import ml_dtypes

import concourse.bacc as bacc
import concourse.tile as tile
import concourse.mybir as mybir
from concourse.bass_utils import run_bass_kernel_spmd

F32 = mybir.dt.float32
F32R = mybir.dt.float32r
BF16 = mybir.dt.bfloat16
AF = mybir.ActivationFunctionType
ALU = mybir.AluOpType

B, T, D_OUT = 64, 8192, 128
KPOS = 2
DT = 1.0 / 240.0
N_CORES = 8
BL = B // N_CORES          # 8 batch elements per core
CH = 512                   # timesteps per chunk
CPB = T // CH              # 16 chunks per batch element
NP = BL * CPB              # 128 chunks = partitions
SLOTS = 32                 # feature-slot stride in P
GT = 4                     # chunks per G-tile
NGT = NP // GT             # 32 G-tiles per core

# Layer-2 operand dtype: BF16 (fast, ~2e-3 rel err) or F32R (~1.5e-4).
L2_DT = BF16
# Layer-1 rhs/weights dtype.
L1_DT = F32R

ALPHA_F, ALPHA_S = 0.8, 0.95

# P slot indices
S_FX = 0         # 0..3  sin(x,k0) sin(x,k1) cos(x,k0) cos(x,k1)
S_FY = 4         # 4..7
S_VX, S_VY, S_SPD, S_DC, S_DS = 8, 9, 10, 11, 12
S_AX, S_AY, S_APAR, S_APERP = 13, 14, 15, 16
S_GATE, S_QF, S_QS = 17, 18, 19
S_ONES = 20
S_X240, S_Y240, S_VX240, S_VY240 = 21, 22, 23, 24
S_ISP, S_TA, S_TB, S_TC = 25, 26, 27, 28
S_STAGE = 30     # 30..31: raw interleaved gaze staging [128, 1024]

_cache = {}


def _np_dt(dt):
    return np.float32 if dt in (F32, F32R) else ml_dtypes.bfloat16


def _build_nc():
    nc = bacc.Bacc("TRN2", target_bir_lowering=False, debug=False,
                   num_devices=N_CORES)

    d_gaze = nc.dram_tensor("gaze", [BL, T, 2], F32, kind="ExternalInput")
    d_W1b = nc.dram_tensor("W1b", [128, 128], L1_DT, kind="ExternalInput")
    d_W2 = nc.dram_tensor("W2", [128, 128], L2_DT, kind="ExternalInput")
    d_ones1 = nc.dram_tensor("ones1", [1, 128], L2_DT, kind="ExternalInput")
    d_b2rep = nc.dram_tensor("b2rep", [1, CH], L2_DT, kind="ExternalInput")
    d_S = nc.dram_tensor("Smat", [128, 128], F32, kind="ExternalInput")
    d_ALPH = nc.dram_tensor("ALPH", [128, 2 * CH], F32, kind="ExternalInput")
    d_APOW = nc.dram_tensor("APOW", [128, 2 * CH], F32, kind="ExternalInput")
    d_SCAL = nc.dram_tensor("SCAL", [128, 11], F32, kind="ExternalInput")
    d_out = nc.dram_tensor("out", [BL, T, D_OUT], F32, kind="ExternalOutput")

    PI = float(np.pi)

    with tile.TileContext(nc) as tc:
        with (
            tc.tile_pool(name="pP", bufs=1) as pP,
            tc.tile_pool(name="pC", bufs=1) as pC,
            tc.tile_pool(name="pG", bufs=3) as pG,
            tc.tile_pool(name="pH", bufs=2) as pH,
            tc.tile_pool(name="pO", bufs=3) as pO,
            tc.tile_pool(name="ps1", bufs=1, space="PSUM") as ps1,
            tc.tile_pool(name="ps2", bufs=1, space="PSUM") as ps2,
        ):
            P = pP.tile([128, SLOTS * CH], F32)

            def sl(i, n=1):
                return P[:, i * CH:(i + n) * CH]

            # constants / weights
            t_W1b = pC.tile([128, 128], L1_DT, tag="W1b")
            nc.sync.dma_start(out=t_W1b[:], in_=d_W1b[:])
            t_W2 = pC.tile([128, 128], L2_DT, tag="W2")
            nc.sync.dma_start(out=t_W2[:], in_=d_W2[:])
            t_ones1 = pC.tile([1, 128], L2_DT, tag="ones1")
            nc.sync.dma_start(out=t_ones1[:], in_=d_ones1[:])
            t_b2rep = pC.tile([1, CH], L2_DT, tag="b2rep")
            nc.sync.dma_start(out=t_b2rep[:], in_=d_b2rep[:])
            t_S = pC.tile([128, 128], F32, tag="Smat")
            nc.sync.dma_start(out=t_S[:], in_=d_S[:])
            t_ALPH = pC.tile([128, 2 * CH], F32, tag="ALPH")
            nc.sync.dma_start(out=t_ALPH[:], in_=d_ALPH[:])
            t_APOW = pC.tile([128, 2 * CH], F32, tag="APOW")
            nc.sync.dma_start(out=t_APOW[:], in_=d_APOW[:])
            t_SCAL = pC.tile([128, 11], F32, tag="SCAL")
            nc.sync.dma_start(out=t_SCAL[:], in_=d_SCAL[:])

            EBxy = pC.tile([128, 2], F32, tag="EBxy")
            EBv = pC.tile([128, 2], F32, tag="EBv")
            EBq = pC.tile([128, 2], F32, tag="EBq")
            Cxy = pC.tile([128, 2], F32, tag="Cxy")
            Cv = pC.tile([128, 2], F32, tag="Cv")
            Cq = pC.tile([128, 2], F32, tag="Cq")

            # ---- phase A: feature planes ----
            stage = sl(S_STAGE, 2)
            nc.sync.dma_start(
                out=stage,
                in_=d_gaze[:].rearrange("b t two -> b (t two)")
                             .rearrange("b (c f) -> (b c) f", f=2 * CH),
            )
            xs = stage.rearrange("p (t two) -> p two t", two=2)
            x_raw, y_raw = xs[:, 0, :], xs[:, 1, :]

            nc.vector.tensor_scalar_mul(sl(S_X240), x_raw, 1.0 / DT)
            nc.vector.tensor_scalar_mul(sl(S_Y240), y_raw, 1.0 / DT)

            nc.vector.memset(sl(S_ONES), 1.0)

            # chunk-boundary carries for v
            nc.vector.tensor_copy(EBxy[:, 0:1], sl(S_X240)[:, CH - 1:CH])
            nc.vector.tensor_copy(EBxy[:, 1:2], sl(S_Y240)[:, CH - 1:CH])
            psA = ps1.tile([128, 2048], F32, tag="ps1")
            nc.tensor.matmul(psA[:, 0:2], t_S[:], EBxy[:], start=True, stop=True)
            nc.vector.tensor_copy(Cxy[:], psA[:, 0:2])

            for s_v, s_c, col in ((S_VX, S_X240, 0), (S_VY, S_Y240, 1)):
                nc.vector.tensor_tensor(
                    sl(s_v)[:, 1:], sl(s_c)[:, 1:], sl(s_c)[:, :-1], ALU.subtract)
                nc.vector.tensor_tensor(
                    sl(s_v)[:, 0:1], sl(s_c)[:, 0:1], Cxy[:, col:col + 1],
                    ALU.subtract)

            # first chunk of each batch element: v[0] = 0 (prepended frame)
            nc.vector.tensor_scalar_mul(
                sl(S_VX)[:, 0:1], sl(S_VX)[:, 0:1], t_SCAL[:, 10:11])
            nc.vector.tensor_scalar_mul(
                sl(S_VY)[:, 0:1], sl(S_VY)[:, 0:1], t_SCAL[:, 10:11])
            nc.vector.tensor_scalar_mul(sl(S_VX240), sl(S_VX), 1.0 / DT)
            nc.vector.tensor_scalar_mul(sl(S_VY240), sl(S_VY), 1.0 / DT)

            nc.vector.tensor_copy(EBv[:, 0:1], sl(S_VX240)[:, CH - 1:CH])
            nc.vector.tensor_copy(EBv[:, 1:2], sl(S_VY240)[:, CH - 1:CH])
            psB = ps1.tile([128, 2048], F32, tag="ps1")
            nc.tensor.matmul(psB[:, 0:2], t_S[:], EBv[:], start=True, stop=True)
            nc.vector.tensor_copy(Cv[:], psB[:, 0:2])

            for s_a, s_c, col in ((S_AX, S_VX240, 0), (S_AY, S_VY240, 1)):
                nc.vector.tensor_tensor(
                    sl(s_a)[:, 1:], sl(s_c)[:, 1:], sl(s_c)[:, :-1], ALU.subtract)
                nc.vector.tensor_tensor(
                    sl(s_a)[:, 0:1], sl(s_c)[:, 0:1], Cv[:, col:col + 1],
                    ALU.subtract)

            # speed, 1/(speed+eps), direction
            nc.vector.tensor_tensor(sl(S_TA), sl(S_VX), sl(S_VX), ALU.mult)
            nc.vector.tensor_tensor(sl(S_TB), sl(S_VY), sl(S_VY), ALU.mult)
            nc.vector.tensor_tensor(sl(S_TA), sl(S_TA), sl(S_TB), ALU.add)
            nc.scalar.activation(sl(S_SPD), sl(S_TA), AF.Sqrt)
            nc.vector.tensor_scalar_add(sl(S_TB), sl(S_SPD), 1e-6)
            nc.vector.reciprocal_approx_accurate(sl(S_ISP), sl(S_TB), sl(S_TA))
            nc.vector.tensor_tensor(sl(S_DC), sl(S_VX), sl(S_ISP), ALU.mult)
            nc.vector.tensor_tensor(sl(S_DS), sl(S_VY), sl(S_ISP), ALU.mult)

            # a_par, a_perp
            nc.vector.tensor_tensor(sl(S_TA), sl(S_VX), sl(S_AX), ALU.mult)
            nc.vector.tensor_tensor(sl(S_TB), sl(S_VY), sl(S_AY), ALU.mult)
            nc.vector.tensor_tensor(sl(S_TA), sl(S_TA), sl(S_TB), ALU.add)
            nc.vector.tensor_tensor(sl(S_APAR), sl(S_TA), sl(S_ISP), ALU.mult)
            nc.vector.tensor_tensor(sl(S_TA), sl(S_VX), sl(S_AY), ALU.mult)
            nc.vector.tensor_tensor(sl(S_TB), sl(S_VY), sl(S_AX), ALU.mult)
            nc.vector.tensor_tensor(sl(S_TA), sl(S_TA), sl(S_TB), ALU.subtract)
            nc.vector.tensor_tensor(sl(S_APERP), sl(S_TA), sl(S_ISP), ALU.mult)

            # gate = sigmoid(invT*speed - invT*thr)
            nc.vector.tensor_scalar(
                sl(S_TA), sl(S_SPD), t_SCAL[:, 8:9], t_SCAL[:, 9:10],
                ALU.mult, ALU.add)
            nc.scalar.activation(sl(S_GATE), sl(S_TA), AF.Sigmoid)

            # EMA scans (within-chunk) + carry fixup
            nc.vector.tensor_scalar_mul(sl(S_TA), sl(S_GATE), 1.0 - ALPHA_F)
            nc.vector.tensor_tensor_scan(
                sl(S_QF), t_ALPH[:, 0:CH], sl(S_TA), 0.0, ALU.mult, ALU.add)
            nc.vector.tensor_scalar_mul(sl(S_TB), sl(S_GATE), 1.0 - ALPHA_S)
            nc.vector.tensor_tensor_scan(
                sl(S_QS), t_ALPH[:, CH:2 * CH], sl(S_TB), 0.0, ALU.mult, ALU.add)
            nc.vector.tensor_copy(EBq[:, 0:1], sl(S_QF)[:, CH - 1:CH])
            nc.vector.tensor_copy(EBq[:, 1:2], sl(S_QS)[:, CH - 1:CH])
            psC = ps1.tile([128, 2048], F32, tag="ps1")
            nc.tensor.matmul(psC[:, 0:2], t_S[:], EBq[:], start=True, stop=True)
            nc.vector.tensor_copy(Cq[:], psC[:, 0:2])
            nc.vector.scalar_tensor_tensor(
                sl(S_QF), t_APOW[:, 0:CH], Cq[:, 0:1], sl(S_QF),
                ALU.mult, ALU.add)
            nc.vector.scalar_tensor_tensor(
                sl(S_QS), t_APOW[:, CH:2 * CH], Cq[:, 1:2], sl(S_QS),
                ALU.mult, ALU.add)

            # fourier features: slots [sin k0, sin k1, cos k0, cos k1] per axis
            for ax_i, (s_base, s_src) in enumerate(
                    ((S_FX, S_X240), (S_FY, S_Y240))):
                for k in range(KPOS):
                    wc = 2 * ax_i + k
                    nc.vector.tensor_scalar(
                        sl(S_TC), sl(s_src), t_SCAL[:, wc:wc + 1],
                        t_SCAL[:, 4 + wc:5 + wc], ALU.mult, ALU.add)
                    nc.vector.add_range_wrap(sl(S_TA), sl(S_TC), 0.0, PI, 2 * PI)
                    nc.scalar.activation(sl(s_base + k), sl(S_TA), AF.Sin)
                    nc.vector.add_range_wrap(
                        sl(S_TB), sl(S_TC), PI / 2, PI, 2 * PI)
                    nc.scalar.activation(
                        sl(s_base + KPOS + k), sl(S_TB), AF.Sin)

            # ---- phase B: per-G-tile matmul pipeline ----
            for i in range(NGT):
                G = pG.tile([128, CH], L1_DT, tag="G")
                nc.gpsimd.dma_start(
                    out=G[:],
                    in_=P[4 * i:4 * i + 4, :].rearrange(
                        "p (s f) -> p s f", s=SLOTS),
                )
                ps_l1 = ps1.tile([128, 2048], F32, tag="ps1")
                for g in range(GT):
                    nc.tensor.matmul(
                        ps_l1[:, CH * g:CH * (g + 1)],
                        t_W1b[32 * g:32 * g + 21, :],
                        G[32 * g:32 * g + 21, :],
                        start=True, stop=True,
                        tile_position=(32 * g, 0),
                    )
                h1 = pH.tile([128, 2048], L2_DT, tag="h1")
                nc.scalar.activation(h1[:], ps_l1[:], AF.Gelu)

                ps_l2 = ps2.tile([128, 2048], F32, tag="ps2")
                for jj in range(GT):
                    nc.tensor.matmul(
                        ps_l2[:, CH * jj:CH * (jj + 1)],
                        t_ones1[:], t_b2rep[:],
                        start=True, stop=False, skip_group_check=True)
                    for j in range(4):
                        o0 = CH * jj + 128 * j
                        nc.tensor.matmul(
                            ps_l2[:, o0:o0 + 128],
                            h1[:, o0:o0 + 128],
                            t_W2[:],
                            start=False, stop=True, skip_group_check=True)
                o_t = pO.tile([128, 2048], F32, tag="o")
                nc.scalar.activation(o_t[:], ps_l2[:], AF.Gelu)

                b = (4 * i) // CPB
                c0 = (4 * i) % CPB
                dst = d_out[b, c0 * CH:(c0 + 4) * CH, :].rearrange(
                    "(g j p) d -> p g j d", g=4, j=4)
                nc.sync.dma_start(
                    out=dst,
                    in_=o_t[:].rearrange("p (g j d) -> p g j d", g=4, j=4))

    nc.compile()
    return nc


def _host_consts(pos_logw_x, pos_phi_x, pos_logw_y, pos_phi_y,
                 sac_log_thr, sac_invT, W1, b1, W2, b2):
    S_np = np.zeros((128, 128), np.float32)
    for p in range(1, 128):
        if p % CPB != 0:
            S_np[p - 1, p] = 1.0

    t = np.arange(CH, dtype=np.float64) + 1.0
    APOW = np.concatenate([ALPHA_F ** t, ALPHA_S ** t]).astype(np.float32)
    APOW = np.broadcast_to(APOW[None, :], (128, 2 * CH)).copy()
    ALPH = np.concatenate([
        np.full(CH, ALPHA_F, np.float32), np.full(CH, ALPHA_S, np.float32)])
    ALPH = np.broadcast_to(ALPH[None, :], (128, 2 * CH)).copy()

    w_x = np.exp(pos_logw_x.astype(np.float64))
    w_y = np.exp(pos_logw_y.astype(np.float64))
    scal = np.zeros(11, np.float64)
    scal[0:2] = 2.0 * math.pi * w_x * DT   # applied to x/dt
    scal[2:4] = 2.0 * math.pi * w_y * DT
    scal[4:6] = pos_phi_x.astype(np.float64)
    scal[6:8] = pos_phi_y.astype(np.float64)
    scal[8] = float(sac_invT)
    scal[9] = -float(sac_invT) * math.exp(float(sac_log_thr))
    SCAL = np.broadcast_to(scal.astype(np.float32)[None, :], (128, 11)).copy()
    SCAL[:, 10] = (np.arange(128) % CPB != 0).astype(np.float32)

    W1b = np.zeros((128, 128), np.float32)
    for g in range(4):
        W1b[32 * g:32 * g + 20, :] = W1
        W1b[32 * g + 20, :] = b1
    np_l2 = _np_dt(L2_DT)
    return {
        "Smat": S_np, "ALPH": ALPH, "APOW": APOW, "SCAL": SCAL,
        "W1b": W1b.astype(_np_dt(L1_DT)),
        "W2": np.asarray(W2, np.float32).astype(np_l2),
        "ones1": np.ones((1, 128), np.float32).astype(np_l2),
        "b2rep": np.tile(np.asarray(b2, np.float32), 4)[None, :].astype(np_l2),
    }


def kernel(gaze_xy, pos_logw_x, pos_phi_x, pos_logw_y, pos_phi_y,
           sac_log_thr, sac_invT, W1, b1, W2, b2, _trace=False, _tmpdir=None):
    if "nc" not in _cache:
        _cache["nc"] = _build_nc()
    nc = _cache["nc"]

    consts = _host_consts(pos_logw_x, pos_phi_x, pos_logw_y, pos_phi_y,
                          sac_log_thr, sac_invT, W1, b1, W2, b2)
    gaze_xy = np.asarray(gaze_xy, np.float32)
    in_maps = []
    for i in range(N_CORES):
        m = dict(consts)
        m["gaze"] = np.ascontiguousarray(gaze_xy[i * BL:(i + 1) * BL])
        in_maps.append(m)

    res = run_bass_kernel_spmd(nc, in_maps, list(range(N_CORES)),
                               trace=_trace, tmpdir=_tmpdir)
    out = np.concatenate([res.results[i]["out"] for i in range(N_CORES)], 0)
    if _trace:
        _cache["last_result"] = res
    return out

